# revision 13
# baseline (speedup 1.0000x reference)
"""Trainium2 Bass kernel for nn_CAKernel_47459388621075.

10 steps of x = clip(x + 0.1*relu(conv5x5_circular(x, W)), 0, 1) on
x:(16,3,1024,1024) f32, W:(3,3,5,5) f32.

Sharding: batch-parallel over 8 NeuronCores (2 images/core) — the circular
conv is per-image, so no cross-core communication is needed at all.

Per-core kernel: each step streams the image through SBUF in 32-row blocks.
A block's input window is 36 row-slots x 3 channels interleaved on 108
partitions (slot k, channel ci -> partition 3k+ci): slots 0..33 hold rows
O0..O0+33, slots 34,35 hold the top halo rows O0-2,O0-1. The 5x5x3x3 conv
is computed as 5 PSUM-accumulated matmuls (one per kernel column dx), each
with a banded stationary matrix lhsT[108,96] that encodes all 3 input
channels x 5 row taps -> 32 output rows x 3 output channels (m = 3r+co).
Matmul dtype float32r (fp32 data, full-rate replicated mode). The banded
matrices are precomputed host-side from W (75 weights -> [108,5,96]).

Update: ACT relu(0.1*y) from PSUM, DVE add x (partition-aligned by
construction), DVE min(.,1) — the lower clip never binds since x >= 0.
"""
import sys

sys.path.insert(0, "/opt/trn_rl_repo")

import numpy as np

N_CORES = 8
B = 32          # output rows per block
NSLOT = 36      # window row-slots (34 affine + 2 top-halo)
KP = 3 * NSLOT  # 108 rhs partitions
MP = KP         # psum partitions (36*co + r, r<32 used)
CG = 512        # matmul column group (PSUM bank)


def make_lhsT(W: np.ndarray, b_rows: int = B) -> np.ndarray:
    """Band matrices: lhsT[p=36*ci+k, dx, m=36*co+r] = W[co,ci,dy,dx].

    Window slot k of channel-band ci holds image row O0+k (k<34) or the top
    halo rows O0-2,O0-1 (k=34,35). Output column m=36*co+r is out row O0+r
    of channel co; columns with r>=32 stay zero (partition gap).
    """
    assert W.shape == (3, 3, 5, 5)
    nslot = b_rows + 4
    lhsT = np.zeros((3 * nslot, 5, 3 * nslot), dtype=np.float32)
    for r in range(b_rows):
        for dy in range(5):
            v = r + dy - 2          # input row offset relative to O0
            slot = v if v >= 0 else nslot + v   # -2 -> nslot-2, -1 -> nslot-1
            for dx in range(5):
                for ci in range(3):
                    for co in range(3):
                        lhsT[nslot * ci + slot, dx, nslot * co + r] = W[co, ci, dy, dx]
    return lhsT.reshape(3 * nslot, 5 * 3 * nslot)


def build_body(tc, x_ap, lhsT_ap, y_ap, n_img, H, Wc, steps):
    """Emit the Tile program. x_ap,(n_img,3,H,Wc) in; y_ap same shape out."""
    import concourse.bass as bass
    from concourse import mybir

    nc = tc.nc
    f32 = mybir.dt.float32
    f32r = mybir.dt.float32r
    Relu = mybir.ActivationFunctionType.Relu

    n_blocks = H // B
    assert H % B == 0 and Wc % 256 == 0
    n_cg = max(1, Wc // CG)
    cg = Wc // n_cg
    WF = Wc + 4  # window free width (cols -2..Wc+1)

    # ping-pong DRAM scratch; even-numbered states land in y which is also
    # the final output (steps is even=10); for odd `steps`, y still holds
    # the final state because the last write targets y by construction.
    xmid = nc.dram_tensor("xmid", (n_img, 3, H, Wc), f32, kind="Internal").ap()
    # buffers[s] = tensor read at step s; write target of step s is buffers[s+1]
    bufs = [x_ap] + [xmid if (s % 2 == 1) else y_ap for s in range(1, steps)] + [y_ap]
    if steps % 2 == 1:
        # make the final write land in y: re-derive chain backwards
        bufs = [x_ap] + [y_ap if (s % 2 == 1) else xmid for s in range(1, steps)] + [y_ap]

    from contextlib import ExitStack

    ctx = ExitStack()
    const_pool = ctx.enter_context(tc.tile_pool(name="const", bufs=1))
    win_pool = ctx.enter_context(tc.tile_pool(name="win", bufs=4))
    act_pool = ctx.enter_context(tc.tile_pool(name="act", bufs=3))
    out_pool = ctx.enter_context(tc.tile_pool(name="out", bufs=3))
    psum_pool = ctx.enter_context(tc.tile_pool(name="psum", bufs=4, space="PSUM"))

    lhsT_t = const_pool.tile([KP, 5 * MP], f32r)
    nc.gpsimd.dma_start(lhsT_t[:], lhsT_ap[:, :].bitcast(f32r))

    for s in range(steps):
        src, dst = bufs[s], bufs[s + 1]
        for img in range(n_img):
            for b in range(n_blocks):
                O0 = b * B
                win = win_pool.tile([KP, WF], f32r)
                # channel band ci at partitions [36ci, 36ci+36): slots 0..33 =
                # rows O0..O0+33, slots 34,35 = top halo rows O0-2,O0-1
                for ci in range(3):
                    p0 = NSLOT * ci
                    if b < n_blocks - 1:
                        nc.gpsimd.dma_start(
                            win[p0 : p0 + B + 2, 2 : Wc + 2],
                            src[img, ci, O0 : O0 + B + 2, :].bitcast(f32r),
                        )
                    else:
                        nc.gpsimd.dma_start(
                            win[p0 : p0 + B, 2 : Wc + 2],
                            src[img, ci, O0 : O0 + B, :].bitcast(f32r),
                        )
                        nc.gpsimd.dma_start(
                            win[p0 + B : p0 + B + 2, 2 : Wc + 2],
                            src[img, ci, 0:2, :].bitcast(f32r),
                        )
                    hr = (O0 - 2) % H
                    nc.gpsimd.dma_start(
                        win[p0 + NSLOT - 2 : p0 + NSLOT, 2 : Wc + 2],
                        src[img, ci, hr : hr + 2, :].bitcast(f32r),
                    )
                # circular column halos
                nc.vector.tensor_copy(win[:, 0:2], win[:, Wc : Wc + 2])
                nc.vector.tensor_copy(win[:, Wc + 2 : Wc + 4], win[:, 2:4])

                psum = psum_pool.tile([MP, Wc], f32)
                for g in range(n_cg):
                    for dx in range(5):
                        nc.tensor.matmul(
                            psum[:, g * cg : (g + 1) * cg],
                            lhsT_t[:, MP * dx : MP * (dx + 1)],
                            win[0:KP, g * cg + dx : g * cg + dx + cg],
                            start=(dx == 0),
                            stop=(dx == 4),
                        )

                t = act_pool.tile([MP, Wc], f32)
                nc.scalar.activation(t[:], psum[:], Relu, scale=0.1)
                xn = out_pool.tile([MP, Wc], f32)
                nc.vector.tensor_add(xn[:], t[:], win[0:MP, 2 : Wc + 2].bitcast(f32))
                nc.vector.tensor_scalar_min(xn[:], xn[:], 1.0)

                for ci in range(3):
                    nc.gpsimd.dma_start(
                        dst[img, ci, O0 : O0 + B, :],
                        xn[NSLOT * ci : NSLOT * ci + B, :],
                    )

    ctx.close()


_PROGRAM_CACHE = {}


def _build_program(n_img, H, Wc, steps):
    key = (n_img, H, Wc, steps)
    if key in _PROGRAM_CACHE:
        return _PROGRAM_CACHE[key]
    import concourse.tile as tile
    from concourse import bacc, mybir

    nc = bacc.Bacc(
        "TRN2",
        target_bir_lowering=False,
        debug=False,
        enable_asserts=False,
        num_devices=N_CORES,
    )
    f32 = mybir.dt.float32
    x_ap = nc.dram_tensor("x", (n_img, 3, H, Wc), f32, kind="ExternalInput").ap()
    lhsT_ap = nc.dram_tensor("lhsT", (KP, 5 * MP), f32, kind="ExternalInput").ap()
    y_ap = nc.dram_tensor("y", (n_img, 3, H, Wc), f32, kind="ExternalOutput").ap()
    with tile.TileContext(nc) as tc:
        build_body(tc, x_ap, lhsT_ap, y_ap, n_img, H, Wc, steps)
    nc.compile()
    _PROGRAM_CACHE[key] = nc
    return nc


def kernel(x: np.ndarray, W: np.ndarray, steps) -> np.ndarray:
    from concourse.bass_utils import run_bass_kernel_spmd

    x = np.ascontiguousarray(np.asarray(x), dtype=np.float32)
    W = np.asarray(W, dtype=np.float32)
    steps = int(steps)
    n, c, H, Wc = x.shape
    assert c == 3 and n % N_CORES == 0
    per = n // N_CORES

    nc = _build_program(per, H, Wc, steps)
    lhsT = make_lhsT(W)
    in_maps = [
        {"x": x[i * per : (i + 1) * per], "lhsT": lhsT} for i in range(N_CORES)
    ]
    res = run_bass_kernel_spmd(nc, in_maps, core_ids=list(range(N_CORES)))
    out = np.concatenate([res.results[i]["y"] for i in range(N_CORES)], axis=0)
    return out.astype(np.float32)
